# revision 1
# baseline (speedup 1.0000x reference)
"""DeepSATConv GNN message-passing kernel for 8 Trainium2 NeuronCores.

Math note: the reference computes a per-channel segment-softmax over
msg = self_h[src] + neib_h[dst].  Within a dst-segment, neib_h[dst] (and
b_self, b_nb) are constant per channel, so they cancel in the softmax.
Hence alpha = segsoftmax(h[src] @ W_self.T) exactly, and
out[n] = segsum(e * h[src]) / segsum(e)  with e = exp((h @ W_self.T)[src]),
falling back to h[n] for zero-in-degree nodes.  W_nb / b_nb / b_self do
not affect the output at all.

Sharding: nodes are split across the 8 cores (2500 each); edges are
partitioned by destination node so segment reductions stay core-local;
h is replicated (the "halo gather" degenerates to replication).

Per core the kernel
  A) computes self_h = h @ W_self.T for all nodes into core-local HBM
     (replicated compute; cheaper than cross-core collectives),
  B) for each 128-node tile, dma_gathers self_h[src] and h[src] for the
     tile's (dst-sorted, padded) edge list, then for each 128-edge chunk
     builds a one-hot selector S[e, n] = (dst_local[e] == n) on the DVE
     and accumulates  [denom | numer] = S.T @ [exp(sh) | exp(sh) * hs]
     into a PSUM bank over all chunks of the tile,
  C) finalizes out = numer / max(denom, tiny), with copy_predicated
     restoring h for empty nodes, and writes the tile to HBM.
"""

import os
import numpy as np

N_NODES = 20000
N_EDGES = 320000
D = 256
CORES = 8
NPC = N_NODES // CORES          # 2500 nodes per core
NT = (NPC + 127) // 128         # 20 node tiles per core
NROWS = NT * 128                # 2560 padded rows per core
NT_ALL = (N_NODES + 127) // 128 + 1   # 158 self_h tiles (padded even count)
NPAD = NT_ALL * 128             # 20224
BB = 3                          # chunks per exp/mult batch

# float32r runs the selector matmul at 4x the fp32 rate but rounds
# operands to ~tf32 precision (~8e-4 output error vs ~3e-5 for fp32).
USE_F32R = os.environ.get("GNN_F32R", "0") == "1"

_cache = {}


def _build(C):
    import concourse.bacc as bacc
    import concourse.mybir as mybir
    from concourse.tile import TileContext

    nc = bacc.Bacc("TRN2")
    f32 = mybir.dt.float32
    mm_dt = mybir.dt.float32r if USE_F32R else f32

    h_d = nc.dram_tensor("h", [N_NODES, D], f32, kind="ExternalInput")
    hT_d = nc.dram_tensor("hT", [128, 2, NPAD], f32, kind="ExternalInput")
    WT_d = nc.dram_tensor("WT", [128, 2, D], f32, kind="ExternalInput")
    iota_d = nc.dram_tensor("iota", [128, 128], f32, kind="ExternalInput")
    idx_d = nc.dram_tensor("idx", [128, NT * 8 * C], mybir.dt.int16, kind="ExternalInput")
    dstl_d = nc.dram_tensor("dstl", [128, NT * C], f32, kind="ExternalInput")
    hown_d = nc.dram_tensor("hown", [NROWS, D], f32, kind="ExternalInput")
    out_d = nc.dram_tensor("out", [NROWS, D], f32, kind="ExternalOutput")

    CAP = 128 * C
    with TileContext(nc) as tc:
        with (
            tc.tile_pool(name="const", bufs=1) as constp,
            tc.tile_pool(name="pha", bufs=3) as pha,
            tc.tile_pool(name="gat", bufs=2) as gat,
            tc.tile_pool(name="wrk", bufs=3) as wrk,
            tc.tile_pool(name="fin", bufs=2) as fin,
            tc.tile_pool(name="psa", bufs=2, space="PSUM") as psa,
            tc.tile_pool(name="psb", bufs=2, space="PSUM") as psb,
            tc.tile_pool(name="dram", bufs=1, space="DRAM") as dramp,
        ):
            selfh_d = dramp.tile([NPAD, D], f32)

            # ---------- phase A: self_h = h @ W_self.T (all nodes) ----------
            WT_sb = constp.tile([128, 2, D], f32)
            nc.sync.dma_start(WT_sb[:, :, :], WT_d[:, :, :])
            for i in range(NT_ALL):
                hT_sb = pha.tile([128, 2, 128], f32, tag="hT")
                nc.sync.dma_start(hT_sb[:, :, :], hT_d[:, :, i * 128:(i + 1) * 128])
                ps = psa.tile([128, D], f32, tag="ps")
                for kb in range(2):
                    nc.tensor.matmul(
                        ps[:, :], hT_sb[:, kb, :], WT_sb[:, kb, :],
                        start=(kb == 0), stop=(kb == 1),
                    )
                sh_sb = pha.tile([128, D], f32, tag="shs")
                nc.scalar.copy(sh_sb[:, :], ps[:, :])
                nc.sync.dma_start(selfh_d[i * 128:(i + 1) * 128, :], sh_sb[:, :])

            # ---------- constants ----------
            iota_sb = constp.tile([128, 128], f32)
            nc.sync.dma_start(iota_sb[:, :], iota_d[:, :])
            dstl_sb = constp.tile([128, NT * C], f32)
            nc.sync.dma_start(dstl_sb[:, :], dstl_d[:, :])
            idx_sb = constp.tile([128, NT * 8 * C], mybir.dt.int16)
            nc.sync.dma_start(idx_sb[:, :], idx_d[:, :])

            # ---------- phase B: per node-tile segment softmax ----------
            for t in range(NT):
                hs_t = gat.tile([128, C, D], f32, tag="hs")
                sh_t = gat.tile([128, C, D], f32, tag="sh")
                isl = idx_sb[:, t * 8 * C:(t + 1) * 8 * C]
                nc.gpsimd.dma_gather(
                    hs_t[:, :, :], h_d[:, :], isl, CAP, CAP, D,
                    single_packet=False,
                )
                nc.gpsimd.dma_gather(
                    sh_t[:, :, :], selfh_d[:, :], isl, CAP, CAP, D,
                    single_packet=False,
                )
                acc = psb.tile([128, 2 * D], f32, tag="acc")
                for g in range((C + BB - 1) // BB):
                    b = min(BB, C - g * BB)
                    eX = wrk.tile([128, BB, 2 * D], mm_dt, tag="eX")
                    nc.scalar.activation(
                        eX[:, 0:b, 0:D], sh_t[:, g * BB:g * BB + b, :],
                        mybir.ActivationFunctionType.Exp,
                    )
                    nc.vector.tensor_tensor(
                        eX[:, 0:b, D:2 * D], eX[:, 0:b, 0:D],
                        hs_t[:, g * BB:g * BB + b, :],
                        mybir.AluOpType.mult,
                    )
                    for j in range(b):
                        k = g * BB + j
                        S = wrk.tile([128, 128], mm_dt, tag="S")
                        nc.vector.tensor_scalar(
                            S[:, :], iota_sb[:, :],
                            dstl_sb[:, t * C + k:t * C + k + 1], None,
                            mybir.AluOpType.is_equal,
                        )
                        nc.tensor.matmul(
                            acc[:, :], S[:, :], eX[:, j, :],
                            start=(k == 0), stop=(k == C - 1),
                        )

                # ---------- finalize tile ----------
                dmax = fin.tile([128, D], f32, tag="dmax")
                nc.vector.tensor_scalar(
                    dmax[:, :], acc[:, 0:D], 1e-37, None, mybir.AluOpType.max
                )
                rec = fin.tile([128, D], f32, tag="rec")
                nc.vector.reciprocal(rec[:, :], dmax[:, :])
                res = fin.tile([128, D], f32, tag="res")
                nc.vector.tensor_tensor(
                    res[:, :], acc[:, D:2 * D], rec[:, :], mybir.AluOpType.mult
                )
                mask = fin.tile([128, D], mybir.dt.uint8, tag="mask")
                nc.vector.tensor_scalar(
                    mask[:, :], acc[:, 0:D], 0.0, None, mybir.AluOpType.is_equal
                )
                hown_sb = fin.tile([128, D], f32, tag="hown")
                nc.sync.dma_start(hown_sb[:, :], hown_d[t * 128:(t + 1) * 128, :])
                nc.vector.copy_predicated(res[:, :], mask[:, :], hown_sb[:, :])
                nc.sync.dma_start(out_d[t * 128:(t + 1) * 128, :], res[:, :])
    nc.compile()
    return nc


def _wrap_idx(ix):
    # dma_gather index layout: logical index i lands at output
    # [partition i%128, slot i//128]; the SBUF index tile stores it at
    # [i%16, 8*(i//128) + (i%128)//16], replicated over the 8 Q7 cores.
    w = ix.astype(np.int16).reshape(-1, 8, 16).transpose(2, 0, 1).reshape(16, -1)
    return np.tile(w, (8, 1))


def kernel(h, W_nb, b_nb, W_self, b_self, src, dst):
    from concourse.bass_utils import run_bass_kernel_spmd

    h = np.ascontiguousarray(np.asarray(h, dtype=np.float32))
    W = np.asarray(W_self, dtype=np.float32)
    src = np.asarray(src, dtype=np.int64)
    dst = np.asarray(dst, dtype=np.int64)

    order = np.argsort(dst, kind="stable")
    src_s = src[order]
    dst_s = dst[order]

    # per-(core, tile) edge ranges; tiles are 128 consecutive owned nodes
    tile_base = []
    for c in range(CORES):
        for t in range(NT):
            tile_base.append(c * NPC + t * 128)
    bounds_lo = np.searchsorted(dst_s, np.array(tile_base), side="left")
    hi_nodes = [min(b + 128, (b // NPC + 1) * NPC) for b in tile_base]
    bounds_hi = np.searchsorted(dst_s, np.array(hi_nodes), side="left")
    cnts = bounds_hi - bounds_lo
    C = max(1, int((cnts.max() + 127) // 128))
    assert C <= 40, f"edge distribution too skewed for SBUF budget (C={C})"
    CAP = 128 * C

    # host-side layout prep
    hT = np.zeros((D, NPAD), dtype=np.float32)
    hT[:, :N_NODES] = h.T
    hT = np.ascontiguousarray(hT.reshape(2, 128, NPAD).transpose(1, 0, 2))
    WT = np.ascontiguousarray(W.T.reshape(2, 128, D).transpose(1, 0, 2))
    iota = np.broadcast_to(np.arange(128, dtype=np.float32), (128, 128)).copy()

    in_maps = []
    for c in range(CORES):
        idx_parts = []
        dstl_parts = []
        for t in range(NT):
            i = c * NT + t
            lo, hi = int(bounds_lo[i]), int(bounds_hi[i])
            spad = np.zeros(CAP, dtype=np.int64)
            spad[: hi - lo] = src_s[lo:hi]
            dl = np.full(CAP, -1.0, dtype=np.float32)
            dl[: hi - lo] = (dst_s[lo:hi] - tile_base[i]).astype(np.float32)
            idx_parts.append(_wrap_idx(spad))
            dstl_parts.append(dl.reshape(C, 128).T)
        hown = np.zeros((NROWS, D), dtype=np.float32)
        hown[:NPC] = h[c * NPC:(c + 1) * NPC]
        in_maps.append({
            "h": h,
            "hT": hT,
            "WT": WT,
            "iota": iota,
            "idx": np.ascontiguousarray(np.concatenate(idx_parts, axis=1)),
            "dstl": np.ascontiguousarray(np.concatenate(dstl_parts, axis=1)),
            "hown": hown,
        })

    if C not in _cache:
        _cache[C] = _build(C)
    nc = _cache[C]

    res = run_bass_kernel_spmd(nc, in_maps, core_ids=list(range(CORES)))
    out = np.concatenate(
        [res.results[c]["out"][:NPC] for c in range(CORES)], axis=0
    )
    return out.astype(np.float32)


# revision 2
# speedup vs baseline: 1.1986x; 1.1986x over previous
"""DeepSATConv GNN message-passing kernel for 8 Trainium2 NeuronCores.

Math note: the reference computes a per-channel segment-softmax over
msg = self_h[src] + neib_h[dst].  Within a dst-segment, neib_h[dst] (and
b_self, b_nb) are constant per channel, so they cancel in the softmax.
Hence alpha = segsoftmax(h[src] @ W_self.T) exactly, and
out[n] = segsum(e * h[src]) / segsum(e)  with e = exp((h @ W_self.T)[src]),
falling back to h[n] for zero-in-degree nodes.  W_nb / b_nb / b_self do
not affect the output at all.

Sharding: nodes are split across the 8 cores (2500 each); edges are
partitioned by destination node so segment reductions stay core-local;
h is replicated (the "halo gather" degenerates to replication).

Per core the kernel
  A) computes Z = h @ [W_self.T | I] = [self_h | h] for all nodes into
     core-local HBM (replicated compute; cheaper than collectives, and
     packing h alongside self_h lets one dma_gather descriptor fetch
     both operands per edge — SWDGE descriptor generation on the Q7 is
     the dominant cost of gathers),
  B) for each 128-node tile, dma_gathers Z[src] for the tile's
     (dst-sorted, padded) edge list, then for each 128-edge chunk
     builds a one-hot selector S[e, n] = (dst_local[e] == n) on the DVE
     and accumulates  [denom | numer] = S.T @ [exp(sh) | exp(sh) * hs]
     into a PSUM bank over all chunks of the tile,
  C) finalizes out = numer / max(denom, tiny), with copy_predicated
     restoring h for empty nodes, and writes the tile to HBM.
"""

import os
import numpy as np

N_NODES = 20000
N_EDGES = 320000
D = 256
CORES = 8
NPC = N_NODES // CORES          # 2500 nodes per core
NT = (NPC + 127) // 128         # 20 node tiles per core
NROWS = NT * 128                # 2560 padded rows per core
NT_ALL = (N_NODES + 127) // 128 + 1   # 158 phase-A tiles (padded even count)
NPAD = NT_ALL * 128             # 20224
BB = 3                          # chunks per exp/mult batch

# float32r runs the selector matmul at 4x the fp32 rate but rounds
# operands to ~tf32 precision (~8e-4 output error vs ~3e-5 for fp32).
USE_F32R = os.environ.get("GNN_F32R", "0") == "1"

_cache = {}


def _build(C):
    import concourse.bacc as bacc
    import concourse.mybir as mybir
    from concourse.tile import TileContext

    nc = bacc.Bacc("TRN2")
    f32 = mybir.dt.float32
    mm_dt = mybir.dt.float32r if USE_F32R else f32

    hT_d = nc.dram_tensor("hT", [128, 2, NPAD], f32, kind="ExternalInput")
    WI_d = nc.dram_tensor("WI", [128, 2, 2 * D], f32, kind="ExternalInput")
    iota_d = nc.dram_tensor("iota", [128, 128], f32, kind="ExternalInput")
    idx_d = nc.dram_tensor("idx", [128, NT * 8 * C], mybir.dt.int16, kind="ExternalInput")
    dstl_d = nc.dram_tensor("dstl", [128, NT * C], f32, kind="ExternalInput")
    hown_d = nc.dram_tensor("hown", [NROWS, D], f32, kind="ExternalInput")
    out_d = nc.dram_tensor("out", [NROWS, D], f32, kind="ExternalOutput")

    CAP = 128 * C
    with TileContext(nc) as tc:
        with (
            tc.tile_pool(name="const", bufs=1) as constp,
            tc.tile_pool(name="pha", bufs=3) as pha,
            tc.tile_pool(name="gat", bufs=2) as gat,
            tc.tile_pool(name="wrk", bufs=3) as wrk,
            tc.tile_pool(name="fin", bufs=2) as fin,
            tc.tile_pool(name="psa", bufs=2, space="PSUM") as psa,
            tc.tile_pool(name="psb", bufs=2, space="PSUM") as psb,
            tc.tile_pool(name="dram", bufs=1, space="DRAM") as dramp,
        ):
            z_d = dramp.tile([NPAD, 2 * D], f32)

            # ---- phase A: Z = h @ [W_self.T | I] = [self_h | h], all nodes ----
            WI_sb = constp.tile([128, 2, 2 * D], f32)
            nc.sync.dma_start(WI_sb[:, :, :], WI_d[:, :, :])
            for i in range(NT_ALL):
                hT_sb = pha.tile([128, 2, 128], f32, tag="hT")
                nc.sync.dma_start(hT_sb[:, :, :], hT_d[:, :, i * 128:(i + 1) * 128])
                ps = psa.tile([128, 2 * D], f32, tag="ps")
                for kb in range(2):
                    nc.tensor.matmul(
                        ps[:, :], hT_sb[:, kb, :], WI_sb[:, kb, :],
                        start=(kb == 0), stop=(kb == 1),
                    )
                z_sb = pha.tile([128, 2 * D], f32, tag="zs")
                nc.scalar.copy(z_sb[:, :], ps[:, :])
                nc.sync.dma_start(z_d[i * 128:(i + 1) * 128, :], z_sb[:, :])

            # ---- constants ----
            iota_sb = constp.tile([128, 128], f32)
            nc.sync.dma_start(iota_sb[:, :], iota_d[:, :])
            dstl_sb = constp.tile([128, NT * C], f32)
            nc.sync.dma_start(dstl_sb[:, :], dstl_d[:, :])
            idx_sb = constp.tile([128, NT * 8 * C], mybir.dt.int16)
            nc.sync.dma_start(idx_sb[:, :], idx_d[:, :])

            # ---- phase B: per node-tile segment softmax ----
            for t in range(NT):
                zx_t = gat.tile([128, C, 2 * D], f32, tag="zx")
                nc.gpsimd.dma_gather(
                    zx_t[:, :, :], z_d[:, :],
                    idx_sb[:, t * 8 * C:(t + 1) * 8 * C], CAP, CAP, 2 * D,
                    single_packet=False,
                )
                acc = psb.tile([128, 2 * D], f32, tag="acc")
                for g in range((C + BB - 1) // BB):
                    b = min(BB, C - g * BB)
                    eX = wrk.tile([128, BB, 2 * D], mm_dt, tag="eX")
                    nc.scalar.activation(
                        eX[:, 0:b, 0:D], zx_t[:, g * BB:g * BB + b, 0:D],
                        mybir.ActivationFunctionType.Exp,
                    )
                    nc.vector.tensor_tensor(
                        eX[:, 0:b, D:2 * D], eX[:, 0:b, 0:D],
                        zx_t[:, g * BB:g * BB + b, D:2 * D],
                        mybir.AluOpType.mult,
                    )
                    for j in range(b):
                        k = g * BB + j
                        S = wrk.tile([128, 128], mm_dt, tag="S")
                        nc.vector.tensor_scalar(
                            S[:, :], iota_sb[:, :],
                            dstl_sb[:, t * C + k:t * C + k + 1], None,
                            mybir.AluOpType.is_equal,
                        )
                        nc.tensor.matmul(
                            acc[:, :], S[:, :], eX[:, j, :],
                            start=(k == 0), stop=(k == C - 1),
                        )

                # ---- finalize tile ----
                dmax = fin.tile([128, D], f32, tag="dmax")
                nc.vector.tensor_scalar(
                    dmax[:, :], acc[:, 0:D], 1e-37, None, mybir.AluOpType.max
                )
                rec = fin.tile([128, D], f32, tag="rec")
                nc.vector.reciprocal(rec[:, :], dmax[:, :])
                res = fin.tile([128, D], f32, tag="res")
                nc.vector.tensor_tensor(
                    res[:, :], acc[:, D:2 * D], rec[:, :], mybir.AluOpType.mult
                )
                mask = fin.tile([128, D], mybir.dt.uint8, tag="mask")
                nc.vector.tensor_scalar(
                    mask[:, :], acc[:, 0:D], 0.0, None, mybir.AluOpType.is_equal
                )
                hown_sb = fin.tile([128, D], f32, tag="hown")
                nc.sync.dma_start(hown_sb[:, :], hown_d[t * 128:(t + 1) * 128, :])
                nc.vector.copy_predicated(res[:, :], mask[:, :], hown_sb[:, :])
                nc.sync.dma_start(out_d[t * 128:(t + 1) * 128, :], res[:, :])
    nc.compile()
    return nc


def _wrap_idx(ix):
    # dma_gather index layout: logical index i lands at output
    # [partition i%128, slot i//128]; the SBUF index tile stores it at
    # [i%16, 8*(i//128) + (i%128)//16], replicated over the 8 Q7 cores.
    w = ix.astype(np.int16).reshape(-1, 8, 16).transpose(2, 0, 1).reshape(16, -1)
    return np.tile(w, (8, 1))


def kernel(h, W_nb, b_nb, W_self, b_self, src, dst):
    from concourse.bass_utils import run_bass_kernel_spmd

    h = np.ascontiguousarray(np.asarray(h, dtype=np.float32))
    W = np.asarray(W_self, dtype=np.float32)
    src = np.asarray(src, dtype=np.int64)
    dst = np.asarray(dst, dtype=np.int64)

    order = np.argsort(dst, kind="stable")
    src_s = src[order]
    dst_s = dst[order]

    # per-(core, tile) edge ranges; tiles are 128 consecutive owned nodes
    tile_base = []
    for c in range(CORES):
        for t in range(NT):
            tile_base.append(c * NPC + t * 128)
    bounds_lo = np.searchsorted(dst_s, np.array(tile_base), side="left")
    hi_nodes = [min(b + 128, (b // NPC + 1) * NPC) for b in tile_base]
    bounds_hi = np.searchsorted(dst_s, np.array(hi_nodes), side="left")
    cnts = bounds_hi - bounds_lo
    C = max(1, int((cnts.max() + 127) // 128))
    assert C <= 36, f"edge distribution too skewed for SBUF budget (C={C})"
    CAP = 128 * C

    # host-side layout prep
    hT = np.zeros((D, NPAD), dtype=np.float32)
    hT[:, :N_NODES] = h.T
    hT = np.ascontiguousarray(hT.reshape(2, 128, NPAD).transpose(1, 0, 2))
    WI = np.zeros((D, 2 * D), dtype=np.float32)
    WI[:, :D] = W.T
    WI[np.arange(D), D + np.arange(D)] = 1.0
    WI = np.ascontiguousarray(WI.reshape(2, 128, 2 * D).transpose(1, 0, 2))
    iota = np.broadcast_to(np.arange(128, dtype=np.float32), (128, 128)).copy()

    in_maps = []
    for c in range(CORES):
        idx_parts = []
        dstl_parts = []
        for t in range(NT):
            i = c * NT + t
            lo, hi = int(bounds_lo[i]), int(bounds_hi[i])
            spad = np.zeros(CAP, dtype=np.int64)
            spad[: hi - lo] = src_s[lo:hi]
            dl = np.full(CAP, -1.0, dtype=np.float32)
            dl[: hi - lo] = (dst_s[lo:hi] - tile_base[i]).astype(np.float32)
            idx_parts.append(_wrap_idx(spad))
            dstl_parts.append(dl.reshape(C, 128).T)
        hown = np.zeros((NROWS, D), dtype=np.float32)
        hown[:NPC] = h[c * NPC:(c + 1) * NPC]
        in_maps.append({
            "hT": hT,
            "WI": WI,
            "iota": iota,
            "idx": np.ascontiguousarray(np.concatenate(idx_parts, axis=1)),
            "dstl": np.ascontiguousarray(np.concatenate(dstl_parts, axis=1)),
            "hown": hown,
        })

    if C not in _cache:
        _cache[C] = _build(C)
    nc = _cache[C]

    res = run_bass_kernel_spmd(nc, in_maps, core_ids=list(range(CORES)))
    out = np.concatenate(
        [res.results[c]["out"][:NPC] for c in range(CORES)], axis=0
    )
    return out.astype(np.float32)


# revision 5
# speedup vs baseline: 1.2023x; 1.0032x over previous
"""DeepSATConv GNN message-passing kernel for 8 Trainium2 NeuronCores.

Math note: the reference computes a per-channel segment-softmax over
msg = self_h[src] + neib_h[dst].  Within a dst-segment, neib_h[dst] (and
b_self, b_nb) are constant per channel, so they cancel in the softmax.
Hence alpha = segsoftmax(h[src] @ W_self.T) exactly, and
out[n] = segsum(e * h[src]) / segsum(e)  with e = exp((h @ W_self.T)[src]),
falling back to h[n] for zero-in-degree nodes.  W_nb / b_nb / b_self do
not affect the output at all.

Sharding: nodes are split across the 8 cores (2500 each); edges are
partitioned by destination node so segment reductions stay core-local;
h is replicated (the "halo gather" degenerates to replication).

Per core the kernel
  A) computes Z = h @ [W_self.T | I] = [self_h | h] for all nodes into
     core-local HBM (replicated compute; cheaper than collectives, and
     packing h alongside self_h lets one dma_gather descriptor fetch
     both operands per edge — SWDGE descriptor generation on the Q7 is
     the dominant cost of gathers),
  B) for each 128-node tile, dma_gathers Z[src] for the tile's
     (dst-sorted, padded) edge list, then for each 128-edge chunk
     builds a one-hot selector S[e, n] = (dst_local[e] == n) on the DVE
     and accumulates  [denom | numer] = S.T @ [exp(sh) | exp(sh) * hs]
     into a PSUM bank over all chunks of the tile,
  C) finalizes out = numer / max(denom, tiny), with copy_predicated
     restoring h for empty nodes, and writes the tile to HBM.
"""

import os
import numpy as np

N_NODES = 20000
N_EDGES = 320000
D = 256
CORES = 8
NPC = N_NODES // CORES          # 2500 nodes per core
NT = (NPC + 127) // 128         # 20 node tiles per core
NROWS = NT * 128                # 2560 padded rows per core
NT_ALL = (N_NODES + 127) // 128 + 1   # 158 phase-A tiles (padded even count)
NPAD = NT_ALL * 128             # 20224
BB = 6                          # chunks per exp/mult batch

# float32r runs the selector matmul at 4x the fp32 rate but rounds
# operands to ~tf32 precision (~8e-4 output error vs ~3e-5 for fp32).
USE_F32R = os.environ.get("GNN_F32R", "0") == "1"

_cache = {}


def _build(C):
    import concourse.bacc as bacc
    import concourse.mybir as mybir
    from concourse.tile import TileContext

    nc = bacc.Bacc("TRN2")
    f32 = mybir.dt.float32
    mm_dt = mybir.dt.float32r if USE_F32R else f32

    bf16 = mybir.dt.bfloat16
    hT_d = nc.dram_tensor("hT", [128, 2, 2, NPAD], bf16, kind="ExternalInput")
    WI_d = nc.dram_tensor("WI", [128, 2, 2, 2 * D], bf16, kind="ExternalInput")
    iota_d = nc.dram_tensor("iota", [128, 128], f32, kind="ExternalInput")
    idx_d = nc.dram_tensor("idx", [128, NT * 8 * C], mybir.dt.int16, kind="ExternalInput")
    dstl_d = nc.dram_tensor("dstl", [128, NT * C], f32, kind="ExternalInput")
    hown_d = nc.dram_tensor("hown", [NROWS, D], f32, kind="ExternalInput")
    out_d = nc.dram_tensor("out", [NROWS, D], f32, kind="ExternalOutput")

    CAP = 128 * C
    with TileContext(nc) as tc:
        with (
            tc.tile_pool(name="const", bufs=1) as constp,
            tc.tile_pool(name="pha", bufs=3) as pha,
            tc.tile_pool(name="gat", bufs=2) as gat,
            tc.tile_pool(name="wrk", bufs=3) as wrk,
            tc.tile_pool(name="fin", bufs=2) as fin,
            tc.tile_pool(name="psa", bufs=2, space="PSUM") as psa,
            tc.tile_pool(name="psb", bufs=2, space="PSUM") as psb,
            tc.tile_pool(name="dram", bufs=1, space="DRAM") as dramp,
        ):
            z_d = dramp.tile([NPAD, 2 * D], f32)

            # ---- phase A: Z = h @ [W_self.T | I] = [self_h | h], all nodes ----
            # bf16 hi/lo split: h = hi + lo, W.T columns split likewise into
            # WI_hi = [W_hi.T | I], WI_lo = [W_lo.T | 0]; three bf16 products
            # hi@WI_hi + hi@WI_lo + lo@WI_hi reproduce fp32 to ~1e-5.
            WI_sb = constp.tile([128, 2, 2, 2 * D], bf16)
            nc.sync.dma_start(WI_sb[:, :, :, :], WI_d[:, :, :, :])
            for i in range(NT_ALL):
                hT_sb = pha.tile([128, 2, 2, 128], bf16, tag="hT")
                nc.sync.dma_start(hT_sb[:, :, :, :], hT_d[:, :, :, i * 128:(i + 1) * 128])
                ps = psa.tile([128, 2 * D], f32, tag="ps")
                nmm = 0
                for hw, ww in ((0, 0), (0, 1), (1, 0)):
                    for kb in range(2):
                        nc.tensor.matmul(
                            ps[:, :], hT_sb[:, hw, kb, :], WI_sb[:, ww, kb, :],
                            start=(nmm == 0), stop=(nmm == 5),
                        )
                        nmm += 1
                z_sb = pha.tile([128, 2 * D], f32, tag="zs")
                nc.scalar.copy(z_sb[:, :], ps[:, :])
                nc.sync.dma_start(z_d[i * 128:(i + 1) * 128, :], z_sb[:, :])

            # ---- constants ----
            iota_sb = constp.tile([128, 128], f32)
            nc.sync.dma_start(iota_sb[:, :], iota_d[:, :])
            dstl_sb = constp.tile([128, NT * C], f32)
            nc.sync.dma_start(dstl_sb[:, :], dstl_d[:, :])
            idx_sb = constp.tile([128, NT * 8 * C], mybir.dt.int16)
            nc.sync.dma_start(idx_sb[:, :], idx_d[:, :])

            # ---- phase B: per node-tile segment softmax ----
            zx_first = {}
            for t in range(NT):
                zx_t = gat.tile([128, C, 2 * D], f32, tag="zx")
                slot = t % 2
                if slot not in zx_first:
                    zx_first[slot] = True
                    nc.vector.memset(zx_t[:, :, :], 0.0)
                nc.gpsimd.dma_gather(
                    zx_t[:, :, :], z_d[:, :],
                    idx_sb[:, t * 8 * C:(t + 1) * 8 * C], CAP, CAP, 2 * D,
                    single_packet=False,
                )
                acc = psb.tile([128, 2 * D], f32, tag="acc")
                for g in range((C + BB - 1) // BB):
                    b = min(BB, C - g * BB)
                    eX = wrk.tile([128, BB, 2 * D], mm_dt, tag="eX")
                    nc.scalar.activation(
                        eX[:, 0:b, 0:D], zx_t[:, g * BB:g * BB + b, 0:D],
                        mybir.ActivationFunctionType.Exp,
                    )
                    nc.vector.tensor_tensor(
                        eX[:, 0:b, D:2 * D], eX[:, 0:b, 0:D],
                        zx_t[:, g * BB:g * BB + b, D:2 * D],
                        mybir.AluOpType.mult,
                    )
                    for j in range(b):
                        k = g * BB + j
                        S = wrk.tile([128, 128], mm_dt, tag="S")
                        nc.vector.tensor_scalar(
                            S[:, :], iota_sb[:, :],
                            dstl_sb[:, t * C + k:t * C + k + 1], None,
                            mybir.AluOpType.is_equal,
                        )
                        nc.tensor.matmul(
                            acc[:, :], S[:, :], eX[:, j, :],
                            start=(k == 0), stop=(k == C - 1),
                        )

                # ---- finalize tile ----
                dmax = fin.tile([128, D], f32, tag="dmax")
                nc.vector.tensor_scalar(
                    dmax[:, :], acc[:, 0:D], 1e-37, None, mybir.AluOpType.max
                )
                rec = fin.tile([128, D], f32, tag="rec")
                nc.vector.reciprocal(rec[:, :], dmax[:, :])
                res = fin.tile([128, D], f32, tag="res")
                nc.vector.tensor_tensor(
                    res[:, :], acc[:, D:2 * D], rec[:, :], mybir.AluOpType.mult
                )
                mask = fin.tile([128, D], mybir.dt.uint8, tag="mask")
                nc.vector.tensor_scalar(
                    mask[:, :], acc[:, 0:D], 0.0, None, mybir.AluOpType.is_equal
                )
                hown_sb = fin.tile([128, D], f32, tag="hown")
                nc.sync.dma_start(hown_sb[:, :], hown_d[t * 128:(t + 1) * 128, :])
                nc.vector.copy_predicated(res[:, :], mask[:, :], hown_sb[:, :])
                nc.sync.dma_start(out_d[t * 128:(t + 1) * 128, :], res[:, :])
    nc.compile()
    return nc


def _wrap_idx(ix):
    # dma_gather index layout: logical index i lands at output
    # [partition i%128, slot i//128]; the SBUF index tile stores it at
    # [i%16, 8*(i//128) + (i%128)//16], replicated over the 8 Q7 cores.
    w = ix.astype(np.int16).reshape(-1, 8, 16).transpose(2, 0, 1).reshape(16, -1)
    return np.tile(w, (8, 1))


def kernel(h, W_nb, b_nb, W_self, b_self, src, dst):
    from concourse.bass_utils import run_bass_kernel_spmd

    h = np.ascontiguousarray(np.asarray(h, dtype=np.float32))
    W = np.asarray(W_self, dtype=np.float32)
    src = np.asarray(src, dtype=np.int64)
    dst = np.asarray(dst, dtype=np.int64)

    order = np.argsort(dst, kind="stable")
    src_s = src[order]
    dst_s = dst[order]

    # per-(core, tile) edge ranges; tiles are 128 consecutive owned nodes
    tile_base = []
    for c in range(CORES):
        for t in range(NT):
            tile_base.append(c * NPC + t * 128)
    bounds_lo = np.searchsorted(dst_s, np.array(tile_base), side="left")
    hi_nodes = [min(b + 128, (b // NPC + 1) * NPC) for b in tile_base]
    bounds_hi = np.searchsorted(dst_s, np.array(hi_nodes), side="left")
    cnts = bounds_hi - bounds_lo
    C = max(1, int((cnts.max() + 127) // 128))
    assert C <= 36, f"edge distribution too skewed for SBUF budget (C={C})"
    CAP = 128 * C

    # host-side layout prep: bf16 hi/lo split of h and W for phase A
    import ml_dtypes
    bf = ml_dtypes.bfloat16
    h_hi = h.astype(bf)
    h_lo = (h - h_hi.astype(np.float32)).astype(bf)
    W_hi = W.astype(bf)
    W_lo = (W - W_hi.astype(np.float32)).astype(bf)

    hT = np.zeros((2, D, NPAD), dtype=bf)
    hT[0, :, :N_NODES] = h_hi.T
    hT[1, :, :N_NODES] = h_lo.T
    # [hw, kb*128+p, n] -> [p, hw, kb, n]
    hT = np.ascontiguousarray(
        hT.reshape(2, 2, 128, NPAD).transpose(2, 0, 1, 3)
    )
    WI = np.zeros((2, D, 2 * D), dtype=bf)
    WI[0, :, :D] = W_hi.T
    WI[1, :, :D] = W_lo.T
    WI[0, np.arange(D), D + np.arange(D)] = bf(1.0)
    WI = np.ascontiguousarray(
        WI.reshape(2, 2, 128, 2 * D).transpose(2, 0, 1, 3)
    )
    iota = np.broadcast_to(np.arange(128, dtype=np.float32), (128, 128)).copy()

    in_maps = []
    for c in range(CORES):
        idx_parts = []
        dstl_parts = []
        for t in range(NT):
            i = c * NT + t
            lo, hi = int(bounds_lo[i]), int(bounds_hi[i])
            spad = np.zeros(CAP, dtype=np.int64)
            spad[: hi - lo] = src_s[lo:hi]
            dl = np.full(CAP, -1.0, dtype=np.float32)
            dl[: hi - lo] = (dst_s[lo:hi] - tile_base[i]).astype(np.float32)
            idx_parts.append(_wrap_idx(spad))
            dstl_parts.append(dl.reshape(C, 128).T)
        hown = np.zeros((NROWS, D), dtype=np.float32)
        hown[:NPC] = h[c * NPC:(c + 1) * NPC]
        in_maps.append({
            "hT": hT,
            "WI": WI,
            "iota": iota,
            "idx": np.ascontiguousarray(np.concatenate(idx_parts, axis=1)),
            "dstl": np.ascontiguousarray(np.concatenate(dstl_parts, axis=1)),
            "hown": hown,
        })

    if C not in _cache:
        _cache[C] = _build(C)
    nc = _cache[C]

    res = run_bass_kernel_spmd(nc, in_maps, core_ids=list(range(CORES)))
    out = np.concatenate(
        [res.results[c]["out"][:NPC] for c in range(CORES)], axis=0
    )
    return out.astype(np.float32)
